# revision 51
# baseline (speedup 1.0000x reference)
"""Bass/Trainium2 kernel for GQA attention (B=1, LQ=LK=2048, D=4096,
H=32, KVH=8, DH=128) distributed over 8 NeuronCores, tensor-parallel by
heads: core i owns kv-head i and its 4 query heads.

Per-core pipeline (all matmuls bf16, accumulation fp32 in PSUM):
  1. qT/kT/v projections from host-transposed hidden states
  2. scoresT = kT . qT per 128k x 512q block (causal blocks only),
     bias+mask added on DVE, exp on ACT
  3. U_T  += v . eT      (unnormalized attention output, transposed)
     S_bc += ones . eT   (row sums broadcast over partitions)
     out_head = U_T * reciprocal(S_bc)
  4. partial_out = attnT . Wo_shard ; host sums the 8 partials
"""
import os
import sys
import types

import numpy as np
import ml_dtypes

sys.path.insert(0, '/opt/trn_rl_repo')

BF16 = ml_dtypes.bfloat16

# ---------------------------------------------------------------- axon shim
def _install_axon_hooks():
    """Provide antenv.axon_hooks (absent in this image) so that
    run_bass_kernel_spmd(trace=True) / BASS_TRACE=1 can capture NTFF
    profiles instead of crashing on import."""
    if "antenv.axon_hooks" in sys.modules:
        return
    state = {"hook": None}
    mod = types.ModuleType("antenv.axon_hooks")
    mod.set_axon_ntff_profile_hook = lambda h: state.__setitem__("hook", h)
    mod.get_axon_ntff_profile_hook = lambda: state["hook"]
    sys.modules["antenv.axon_hooks"] = mod
    try:
        from trn_agent_boot.trn_boot import _ntff_profile_via_ctypes
        mod.set_axon_ntff_profile_hook(
            _ntff_profile_via_ctypes('/opt/axon/libaxon_pjrt.so'))
    except Exception:
        pass


_install_axon_hooks()

import concourse.bass as bass
import concourse.tile as tile
from concourse.tile import add_dep_helper
from concourse import mybir
from concourse.bass_utils import run_bass_kernel_spmd
from concourse.alu_op_type import AluOpType
from concourse.masks import make_identity

# ---------------------------------------------------------------- constants
B, LQ, LK = 1, 2048, 2048
D, H, KVH, DH = 4096, 32, 8, 128
G = H // KVH          # 4 query heads per kv head
N_CORES = 8
NH = H // N_CORES     # 4 heads per core
KO = D // 128         # 32 contraction chunks for the projections
QC = 512              # q free-dim chunk for attention blocks
NEG = -30000.0        # additive mask value (exp -> exactly 0 in fp32)

FP32 = mybir.dt.float32
DT = mybir.dt.bfloat16


def _split_drain_tile_context():
    """TileContext whose final drain splits its semaphore waits across
    multiple drain instructions — walrus in this container rejects CTRL
    instructions carrying more than one sync wait."""
    import bass_rust

    class SplitDrainTC(tile.TileContext):
        def _drain_and_barrier(self, tick_clock, wait_clock):
            drain_inst = self.nc.sync.drain()
            wait_clock.add_sem_waits(
                drain_inst.ins, tile.ScopedClock({None: tick_clock.global_clock})
            )
            si = drain_inst.ins.sync_info
            if si is not None and si.on_wait and len(si.on_wait) > 1:
                waits = list(si.on_wait)
                si.on_wait = waits[:1]
                drain_inst.ins.sync_info = si
                for w in waits[1:]:
                    d2 = self.nc.sync.drain()
                    d2.ins.sync_info = bass_rust.SyncInfo(on_wait=[w], on_update=[])

            self.nc.all_engine_barrier()
            assert self.sems is not None
            popped = self.nc._tile_sem_poison_stack.pop()
            assert popped is self._sem_poison
            self.nc.clear_and_free_semaphores(list(self.sems.allocated().values()))
            self.nc.all_engine_barrier()

    return SplitDrainTC


def build_graph(nk_per_qc, off_table):
    """Build the single-core SPMD graph. nk_per_qc[qc] = number of 128-wide
    key chunks to process for query chunk qc; off_table[qc][kc] = first live
    q column (multiple of 128) of block (kc, qc). Both derived from the
    attention mask on the host."""
    nc = bass.Bass("TRN2", target_bir_lowering=False, debug=False,
                   num_devices=N_CORES)

    hqT = nc.dram_tensor("hqT", [D, LQ], DT, kind="ExternalInput").ap()
    hkvT = nc.dram_tensor("hkvT", [D, LK], DT, kind="ExternalInput").ap()
    wq = nc.dram_tensor("wq", [D, NH * DH], DT, kind="ExternalInput").ap()
    wk = nc.dram_tensor("wk", [D, DH], DT, kind="ExternalInput").ap()
    wv = nc.dram_tensor("wv", [D, DH], DT, kind="ExternalInput").ap()
    wo = nc.dram_tensor("wo", [NH * DH, D], DT, kind="ExternalInput").ap()
    biasT = nc.dram_tensor("biasT", [NH, LK, LQ], DT, kind="ExternalInput").ap()
    out = nc.dram_tensor("out", [LQ, D], DT, kind="ExternalOutput").ap()

    n_s = LQ // QC        # 4 query chunks of 512
    n_m = LQ // 128       # 16 seq chunks of 128

    TC = _split_drain_tile_context()
    with TC(nc) as tc:
        with tc.tile_pool(name="weights", bufs=1) as wpool, \
             tc.tile_pool(name="persist", bufs=1) as ppool, \
             tc.tile_pool(name="bias", bufs=3) as bias_pool:
            ones_sb = wpool.tile([128, 128], DT)
            nc.vector.memset(ones_sb[:], 1.0)
            ident_sb = wpool.tile([128, 128], DT)
            # preload the exp table set while projections run
            warm_sb = wpool.tile([128, 1], FP32)
            nc.scalar.activation(out=warm_sb[:], in_=ones_sb[:, 0:1],
                                 func=mybir.ActivationFunctionType.Exp)


            bias_tiles = {}

            def fetch_bias(qc, h, not_before=None):
                nk = nk_per_qc[qc]
                bias_sb = bias_pool.tile([128, LK // 128, QC], DT,
                                         tag="bias", name=f"bias_{qc}_{h}")
                for g in range((nk + 3) // 4):
                    k0, k1 = g * 4, min(nk, g * 4 + 4)
                    dma = nc.gpsimd.dma_start(
                        out=bias_sb[:, k0:k1, :],
                        in_=biasT[h, k0 * 128:k1 * 128,
                                  qc * QC:(qc + 1) * QC].rearrange(
                            "(ko p) q -> p ko q", p=128))
                    if not_before is not None:
                        add_dep_helper(dma.ins, not_before,
                                       reason="keep bias off startup HBM")
                bias_tiles[(qc, h)] = bias_sb

            # persistent activations
            wo_sb = ppool.tile([128, NH, D], DT)      # [hd_in, h, d_out]
            qT_sb = ppool.tile([128, NH, LQ], DT)     # [dh, h, q]
            kT_sb = ppool.tile([128, LK], DT)         # [dh, k]
            v_sb = ppool.tile([128, LK // 128, DH], DT)   # [k_in, k_blk, dh]
            un_sb = ppool.tile([128, NH, LQ], DT)     # normalized U_T

            # ---------------- stage 1: projections ----------------
            with tc.tile_pool(name="w1", bufs=1) as w1pool, \
                 tc.tile_pool(name="slab", bufs=4) as slab_pool, \
                 tc.tile_pool(name="proj_ps", bufs=3, space="PSUM") as proj_ps, \
                 tc.tile_pool(name="vtr_ps", bufs=2, space="PSUM") as vtr_ps, \
                 nc.named_scope("proj"):
                # all weights ride the SP HWDGE ring (in need order) while
                # the SWDGE ring carries only hidden-state slabs and bias
                wk_sb = w1pool.tile([128, KO, DH], DT)
                nc.sync.dma_start(
                    out=wk_sb[:], in_=wk.rearrange("(ko p) d -> p ko d", p=128))
                wv_sb = w1pool.tile([128, KO, DH], DT)
                nc.sync.dma_start(
                    out=wv_sb[:], in_=wv.rearrange("(ko p) d -> p ko d", p=128))
                vT_sb = w1pool.tile([128, LK], DT)        # [dh, k]
                wq_sb = w1pool.tile([128, KO, NH * DH], DT)
                wq_dmas = []
                for g in range(4):
                    wq_dmas.append(nc.sync.dma_start(
                        out=wq_sb[:, g * 8:(g + 1) * 8, :],
                        in_=wq[g * 1024:(g + 1) * 1024, :].rearrange(
                            "(ko p) d -> p ko d", p=128)))
                # interleaved kv/q steps: spreads the SWDGE slab traffic
                # evenly so the ring never starves the PE
                early_markers = {}

                def kv_step(s):
                    kt_ps = proj_ps.tile([128, QC], FP32, tag="pps",
                                         name=f"ktps_{s}")
                    vt_ps = proj_ps.tile([128, QC], FP32, tag="pps2",
                                         name=f"vtps_{s}")
                    for quarter in range(4):
                        slab = slab_pool.tile([128, KO // 4, QC], DT,
                                              tag="slab", name=f"kvslab_{s}_{quarter}")
                        r0 = quarter * 1024
                        dma = nc.gpsimd.dma_start(
                            out=slab[:],
                            in_=hkvT[r0:r0 + 1024,
                                     s * QC:(s + 1) * QC].rearrange(
                                "(ko p) q -> p ko q", p=128))
                        if s == 0 and quarter == 2:
                            early_markers['kv0q2'] = dma.ins
                        for kl in range(KO // 4):
                            ko = quarter * (KO // 4) + kl
                            nc.tensor.matmul(kt_ps[:], lhsT=wk_sb[:, ko, :],
                                             rhs=slab[:, kl, :],
                                             start=(ko == 0), stop=(ko == KO - 1))
                        for kl in range(KO // 4):
                            ko = quarter * (KO // 4) + kl
                            nc.tensor.matmul(vt_ps[:], lhsT=wv_sb[:, ko, :],
                                             rhs=slab[:, kl, :],
                                             start=(ko == 0), stop=(ko == KO - 1))
                    nc.scalar.copy(out=kT_sb[:, s * QC:(s + 1) * QC], in_=kt_ps[:])
                    return nc.scalar.copy(out=vT_sb[:, s * QC:(s + 1) * QC],
                                          in_=vt_ps[:])

                def q_step(s):
                    q_pss = [proj_ps.tile([128, QC], FP32,
                                          tag=("pps" if h % 2 == 0 else "pps2"),
                                          name=f"qps_{s}_{h}")
                             for h in range(NH)]
                    for quarter in range(4):
                        slab = slab_pool.tile([128, KO // 4, QC], DT,
                                              tag="slab", name=f"qslab_{s}_{quarter}")
                        r0 = quarter * 1024
                        nc.gpsimd.dma_start(
                            out=slab[:],
                            in_=hqT[r0:r0 + 1024,
                                    s * QC:(s + 1) * QC].rearrange(
                                "(ko p) q -> p ko q", p=128))
                        for h in range(NH):
                            for kl in range(KO // 4):
                                ko = quarter * (KO // 4) + kl
                                nc.tensor.matmul(
                                    q_pss[h][:],
                                    lhsT=wq_sb[:, ko, h * DH:(h + 1) * DH],
                                    rhs=slab[:, kl, :],
                                    start=(ko == 0), stop=(ko == KO - 1))
                    for h in range(NH):
                        nc.scalar.copy(out=qT_sb[:, h, s * QC:(s + 1) * QC],
                                       in_=q_pss[h][:])

                def v_transpose():
                    make_identity(nc, ident_sb[:])
                    for blk in range(LK // 128):
                        tp = vtr_ps.tile([128, 128], DT, tag="vtr",
                                         name=f"vtr_{blk}")
                        nc.tensor.transpose(
                            tp[:], vT_sb[:, blk * 128:(blk + 1) * 128],
                            ident_sb[:])
                        nc.scalar.copy(out=v_sb[:, blk, :], in_=tp[:])

                kv_step(0)
                for dma in wq_dmas:
                    add_dep_helper(dma.ins, early_markers['kv0q2'],
                                   reason="keep wq off the startup HBM burst")
                q_step(0)
                kv_step(1)
                q_step(1)
                kv_step(2)
                q_step(2)
                marker = kv_step(3).ins
                v_transpose()
                for h in range(NH):
                    # wo/bias are not needed until the attention phase; the
                    # explicit dep keeps the scheduler from hoisting their
                    # transfers into the HBM-critical startup window
                    dma = nc.sync.dma_start(
                        out=wo_sb[:, h, :],
                        in_=wo[h * 128:(h + 1) * 128, :].rearrange(
                            "(hh p) d -> p hh d", p=128))
                    add_dep_helper(dma.ins, marker,
                                   reason="keep wo off startup HBM")
                for h in range(NH):
                    fetch_bias(0, h, not_before=marker)
                q_step(3)

            # ------- stage 2+3: attention interleaved with out-proj -------
            # unnormalized U_T and row-sums staged to SBUF by ACT so the
            # reciprocal/normalize never sits on the DVE critical path

            inv2 = float(DH ** -0.5)
            with tc.tile_pool(name="att_sb", bufs=6) as att_sb, \
                 tc.tile_pool(name="us_sb", bufs=8) as us_pool, \
                 tc.tile_pool(name="sc_ps", bufs=3, space="PSUM") as sc_pool, \
                 tc.tile_pool(name="acc_ps", bufs=2, space="PSUM") as acc_pool, \
                 tc.tile_pool(name="osb", bufs=2) as out_pool, \
                 tc.tile_pool(name="ops", bufs=2, space="PSUM") as out_ps, \
                 nc.named_scope("attn_wo"):

                def emit_attn_unit(qc, h, filler=None):
                    nk = nk_per_qc[qc]
                    if (qc, h) not in bias_tiles:
                        fetch_bias(qc, h)
                    bias_sb = bias_tiles.pop((qc, h))
                    u_ps = acc_pool.tile([128, QC], FP32, tag="ups",
                                         name=f"ups_{qc}_{h}")
                    s_ps = acc_pool.tile([128, QC], FP32, tag="sps", bufs=1,
                                         name=f"sps_{qc}_{h}")
                    e_tiles = {}

                    def off_of(kc):
                        # columns left of the first live one are fully
                        # masked for this key block — skip them
                        return off_table[qc][kc]

                    # software pipeline: score/bias/exp for kc emitted before
                    # the accumulation matmuls of kc-1
                    for kc in range(nk + 1):
                        if kc < nk:
                            off = off_of(kc)
                            sc_ps = sc_pool.tile([128, QC], FP32, tag="scps",
                                                 name=f"scps_{qc}_{h}_{kc}")
                            nc.tensor.matmul(
                                sc_ps[:, off:],
                                lhsT=kT_sb[:, kc * 128:(kc + 1) * 128],
                                rhs=qT_sb[:, h, qc * QC + off:(qc + 1) * QC],
                                start=True, stop=True)
                            t_sb = att_sb.tile([128, QC], FP32, tag="tsb",
                                               name=f"tsb_{qc}_{h}_{kc}")
                            # t = score * dh^-0.5 + bias
                            nc.vector.scalar_tensor_tensor(
                                out=t_sb[:, off:], in0=sc_ps[:, off:],
                                scalar=inv2,
                                in1=bias_sb[:, kc, off:],
                                op0=AluOpType.mult,
                                op1=AluOpType.add)
                            e_sb = att_sb.tile([128, QC], DT, tag="esb",
                                               name=f"esb_{qc}_{h}_{kc}")
                            nc.scalar.activation(
                                out=e_sb[:, off:], in_=t_sb[:, off:],
                                func=mybir.ActivationFunctionType.Exp)
                            e_tiles[kc] = e_sb
                        if kc == min(1, nk - 1) and filler is not None:
                            # PE chews the out-projection chunk here while
                            # DVE/ACT produce the first exp tiles — hides the
                            # score->exp latency at the unit boundary
                            filler()
                        if kc >= 1:
                            off = off_of(kc - 1)
                            e_prev = e_tiles.pop(kc - 1)
                            nc.tensor.matmul(u_ps[:, off:],
                                             lhsT=v_sb[:, kc - 1, :],
                                             rhs=e_prev[:, off:],
                                             start=(kc == 1), stop=(kc == nk),
                                             skip_group_check=True)
                            nc.tensor.matmul(s_ps[:, off:], lhsT=ones_sb[:],
                                             rhs=e_prev[:, off:],
                                             start=(kc == 1), stop=(kc == nk),
                                             skip_group_check=True)
                    nxt = (qc, h + 1) if h + 1 < NH else (qc + 1, 0)
                    if nxt[0] < n_s and nxt not in bias_tiles and nxt != (0, 0):
                        fetch_bias(*nxt)
                    u_sb = us_pool.tile([128, QC], DT, tag="usb",
                                        name=f"usb_{qc}_{h}")
                    s_sb = us_pool.tile([128, QC], DT, tag="ssb",
                                        name=f"ssb_{qc}_{h}")
                    nc.scalar.copy(out=u_sb[:], in_=u_ps[:])
                    nc.scalar.copy(out=s_sb[:], in_=s_ps[:])
                    us_tiles[(qc, h)] = (u_sb, s_sb)

                def emit_norm_piece(qc, h):
                    qsl = slice(qc * QC, (qc + 1) * QC)
                    u_sb, s_sb = us_tiles.pop((qc, h))
                    sinv_sb = att_sb.tile([128, QC], DT, tag="sinv",
                                          name=f"sinv_{qc}_{h}")
                    # 1/S as exp(-ln S) on ACT: Ln and Exp share the
                    # natural_log_exp table set (no table switch), and the
                    # DVE iterative reciprocal would stall the STT pipe
                    lns_sb = att_sb.tile([128, QC], FP32, tag="lns",
                                         name=f"lns_{qc}_{h}")
                    nc.scalar.activation(
                        out=lns_sb[:], in_=s_sb[:],
                        func=mybir.ActivationFunctionType.Ln)
                    nc.scalar.activation(
                        out=sinv_sb[:], in_=lns_sb[:],
                        func=mybir.ActivationFunctionType.Exp,
                        scale=-1.0)
                    nc.vector.tensor_mul(
                        un_sb[:, h, qsl], u_sb[:], sinv_sb[:])

                def emit_wo_unit(m):
                    o_sb = out_pool.tile([128, D], DT, tag="osb",
                                         name=f"osb_{m}")
                    for n in range(D // QC):
                        o_ps = out_ps.tile([128, QC], FP32, tag="ops",
                                           name=f"ops_{m}_{n}")
                        for h in range(NH):
                            nc.tensor.matmul(
                                o_ps[:],
                                lhsT=un_sb[:, h, m * 128:(m + 1) * 128],
                                rhs=wo_sb[:, h, n * QC:(n + 1) * QC],
                                start=(h == 0), stop=(h == NH - 1))
                        nc.scalar.copy(out=o_sb[:, n * QC:(n + 1) * QC],
                                       in_=o_ps[:])
                    nc.scalar.dma_start(out=out[m * 128:(m + 1) * 128, :],
                                        in_=o_sb[:])

                # interleave: attention units stream with 1-unit-delayed
                # norm pieces (keeps the reciprocal off the DVE critical
                # path); wo chunks for qc are woven in as soon as norm(qc)
                # completes, two per unit during the final qc to shrink the
                # tail
                us_tiles = {}
                wo_queue = []
                units = [(qc, h) for qc in range(n_s) for h in range(NH)]
                for i, (qc, h) in enumerate(units):
                    def filler():
                        if wo_queue:
                            emit_wo_unit(wo_queue.pop(0))
                    emit_attn_unit(qc, h, filler=filler if i >= 1 else None)
                    if i >= 1:
                        pqc, ph = units[i - 1]
                        emit_norm_piece(pqc, ph)
                        if ph == NH - 1:
                            wo_queue.extend(range(4 * pqc, 4 * pqc + 4))
                    if qc == n_s - 1 and wo_queue:
                        emit_wo_unit(wo_queue.pop(0))
                emit_norm_piece(*units[-1])
                wo_queue.extend(range(4 * (n_s - 1), 4 * n_s))
                for m in wo_queue:
                    emit_wo_unit(m)
    _split_waits(nc)
    return nc


def _split_waits(nc):
    """Walrus in this container accepts at most one sync wait per
    instruction: hoist extra waits onto same-engine nops placed directly
    before the instruction (identical semantics — the engine stream
    blocks on each in order)."""
    import bass_rust
    ctr = 0
    for f in nc.m.functions:
        for bb in f.blocks:
            new = []
            for inst in bb.instructions:
                si = inst.sync_info
                if si is not None and si.on_wait and len(si.on_wait) > 1:
                    waits = list(si.on_wait)
                    for w in waits[:-1]:
                        nop = bass_rust.InstNoOp(name=f"waitnop-{ctr}",
                                                 engine=inst.engine)
                        ctr += 1
                        nop.sync_info = bass_rust.SyncInfo(on_wait=[w],
                                                           on_update=[])
                        new.append(nop)
                    si.on_wait = waits[-1:]
                    inst.sync_info = si
                new.append(inst)
            bb.instructions = new


_CACHE = {}


def kernel(hidden_q, hidden_kv, attention_mask, position_bias, Wq, Wk, Wv, Wo):
    hq = np.asarray(hidden_q, dtype=np.float32)[0]      # [2048, 4096]
    hkv = np.asarray(hidden_kv, dtype=np.float32)[0]
    mask = np.asarray(attention_mask)[0]                # [2048, 2048] bool
    pb = np.asarray(position_bias, dtype=np.float32)    # [32, 2048, 2048]
    Wq = np.asarray(Wq, dtype=np.float32)
    Wk = np.asarray(Wk, dtype=np.float32)
    Wv = np.asarray(Wv, dtype=np.float32)
    Wo = np.asarray(Wo, dtype=np.float32)

    # additive mask, transposed to [k, q]
    negT = np.where(mask, np.float32(0.0), np.float32(NEG)).T

    # which 128-key chunks are live for each 512-query chunk
    n_s = LQ // QC
    nk_per_qc = []
    off_table = []
    for qc in range(n_s):
        cols = negT[:, qc * QC:(qc + 1) * QC]            # [2048k, 512q]
        live = 0
        offs = []
        for kc in range(LK // 128):
            blk_live = np.any(cols[kc * 128:(kc + 1) * 128] != np.float32(NEG),
                              axis=0)                    # per q column
            if blk_live.any() and kc > 0:
                live = kc + 1
                offs.append(int(np.argmax(blk_live)) // 128 * 128)
            else:
                # kc == 0 must write the full psum width (accumulator init)
                live = max(live, kc + 1) if blk_live.any() else live
                offs.append(0)
        nk_per_qc.append(live)
        off_table.append(offs)
    key = (tuple(nk_per_qc), tuple(tuple(o) for o in off_table))

    if key not in _CACHE:
        _CACHE[key] = build_graph(nk_per_qc, off_table)
    nc = _CACHE[key]

    hqT = np.ascontiguousarray(hq.T).astype(BF16)        # [4096, 2048]
    hkvT = np.ascontiguousarray(hkv.T).astype(BF16)

    in_maps = []
    for i in range(N_CORES):
        bT = np.transpose(pb[NH * i:NH * (i + 1)], (0, 2, 1))  # [4, k, q]
        biasT = (bT + negT[None]).astype(BF16)
        in_maps.append({
            "hqT": hqT,
            "hkvT": hkvT,
            "wq": np.ascontiguousarray(Wq[:, i * NH * DH:(i + 1) * NH * DH]).astype(BF16),
            "wk": np.ascontiguousarray(Wk[:, i * DH:(i + 1) * DH]).astype(BF16),
            "wv": np.ascontiguousarray(Wv[:, i * DH:(i + 1) * DH]).astype(BF16),
            "wo": np.ascontiguousarray(Wo[i * NH * DH:(i + 1) * NH * DH, :]).astype(BF16),
            "biasT": biasT,
        })

    res = run_bass_kernel_spmd(nc, in_maps, list(range(N_CORES)))
    kernel.last_results = res

    acc = np.zeros((LQ, D), dtype=np.float32)
    for i in range(N_CORES):
        acc += res.results[i]["out"].astype(np.float32)
    return acc[None]


# revision 53
# speedup vs baseline: 1.0060x; 1.0060x over previous
"""Bass/Trainium2 kernel for GQA attention (B=1, LQ=LK=2048, D=4096,
H=32, KVH=8, DH=128) distributed over 8 NeuronCores, tensor-parallel by
heads: core i owns kv-head i and its 4 query heads.

Per-core pipeline (all matmuls bf16, accumulation fp32 in PSUM):
  1. qT/kT/v projections from host-transposed hidden states
  2. scoresT = kT . qT per 128k x 512q block (causal blocks only),
     bias+mask added on DVE, exp on ACT
  3. U_T  += v . eT      (unnormalized attention output, transposed)
     S_bc += ones . eT   (row sums broadcast over partitions)
     out_head = U_T * reciprocal(S_bc)
  4. partial_out = attnT . Wo_shard ; host sums the 8 partials
"""
import os
import sys
import types

import numpy as np
import ml_dtypes

sys.path.insert(0, '/opt/trn_rl_repo')

BF16 = ml_dtypes.bfloat16

# ---------------------------------------------------------------- axon shim
def _install_axon_hooks():
    """Provide antenv.axon_hooks (absent in this image) so that
    run_bass_kernel_spmd(trace=True) / BASS_TRACE=1 can capture NTFF
    profiles instead of crashing on import."""
    if "antenv.axon_hooks" in sys.modules:
        return
    state = {"hook": None}
    mod = types.ModuleType("antenv.axon_hooks")
    mod.set_axon_ntff_profile_hook = lambda h: state.__setitem__("hook", h)
    mod.get_axon_ntff_profile_hook = lambda: state["hook"]
    sys.modules["antenv.axon_hooks"] = mod
    try:
        from trn_agent_boot.trn_boot import _ntff_profile_via_ctypes
        mod.set_axon_ntff_profile_hook(
            _ntff_profile_via_ctypes('/opt/axon/libaxon_pjrt.so'))
    except Exception:
        pass


_install_axon_hooks()

import concourse.bass as bass
import concourse.tile as tile
from concourse.tile import add_dep_helper
from concourse import mybir
from concourse.bass_utils import run_bass_kernel_spmd
from concourse.alu_op_type import AluOpType
from concourse.masks import make_identity

# ---------------------------------------------------------------- constants
B, LQ, LK = 1, 2048, 2048
D, H, KVH, DH = 4096, 32, 8, 128
G = H // KVH          # 4 query heads per kv head
N_CORES = 8
NH = H // N_CORES     # 4 heads per core
KO = D // 128         # 32 contraction chunks for the projections
QC = 512              # q free-dim chunk for attention blocks
NEG = -30000.0        # additive mask value (exp -> exactly 0 in fp32)

FP32 = mybir.dt.float32
DT = mybir.dt.bfloat16


def _split_drain_tile_context():
    """TileContext whose final drain splits its semaphore waits across
    multiple drain instructions — walrus in this container rejects CTRL
    instructions carrying more than one sync wait."""
    import bass_rust

    class SplitDrainTC(tile.TileContext):
        def _drain_and_barrier(self, tick_clock, wait_clock):
            drain_inst = self.nc.sync.drain()
            wait_clock.add_sem_waits(
                drain_inst.ins, tile.ScopedClock({None: tick_clock.global_clock})
            )
            si = drain_inst.ins.sync_info
            if si is not None and si.on_wait and len(si.on_wait) > 1:
                waits = list(si.on_wait)
                si.on_wait = waits[:1]
                drain_inst.ins.sync_info = si
                for w in waits[1:]:
                    d2 = self.nc.sync.drain()
                    d2.ins.sync_info = bass_rust.SyncInfo(on_wait=[w], on_update=[])

            self.nc.all_engine_barrier()
            assert self.sems is not None
            popped = self.nc._tile_sem_poison_stack.pop()
            assert popped is self._sem_poison
            self.nc.clear_and_free_semaphores(list(self.sems.allocated().values()))
            self.nc.all_engine_barrier()

    return SplitDrainTC


def build_graph(nk_per_qc, off_table):
    """Build the single-core SPMD graph. nk_per_qc[qc] = number of 128-wide
    key chunks to process for query chunk qc; off_table[qc][kc] = first live
    q column (multiple of 128) of block (kc, qc). Both derived from the
    attention mask on the host."""
    nc = bass.Bass("TRN2", target_bir_lowering=False, debug=False,
                   num_devices=N_CORES)

    hqT = nc.dram_tensor("hqT", [D, LQ], DT, kind="ExternalInput").ap()
    hkvT = nc.dram_tensor("hkvT", [D, LK], DT, kind="ExternalInput").ap()
    wq = nc.dram_tensor("wq", [D, NH * DH], DT, kind="ExternalInput").ap()
    wk = nc.dram_tensor("wk", [D, DH], DT, kind="ExternalInput").ap()
    wv = nc.dram_tensor("wv", [D, DH], DT, kind="ExternalInput").ap()
    wo = nc.dram_tensor("wo", [NH * DH, D], DT, kind="ExternalInput").ap()
    biasT = nc.dram_tensor("biasT", [NH, LK, LQ], DT, kind="ExternalInput").ap()
    out = nc.dram_tensor("out", [LQ, D], DT, kind="ExternalOutput").ap()

    n_s = LQ // QC        # 4 query chunks of 512
    n_m = LQ // 128       # 16 seq chunks of 128

    TC = _split_drain_tile_context()
    with TC(nc) as tc:
        with tc.tile_pool(name="weights", bufs=1) as wpool, \
             tc.tile_pool(name="persist", bufs=1) as ppool, \
             tc.tile_pool(name="bias", bufs=3) as bias_pool:
            ones_sb = wpool.tile([128, 128], DT)
            nc.vector.memset(ones_sb[:], 1.0)
            ident_sb = wpool.tile([128, 128], DT)
            # preload the exp table set while projections run
            warm_sb = wpool.tile([128, 1], FP32)
            nc.scalar.activation(out=warm_sb[:], in_=ones_sb[:, 0:1],
                                 func=mybir.ActivationFunctionType.Exp)


            bias_tiles = {}

            def fetch_bias(qc, h, not_before=None):
                nk = nk_per_qc[qc]
                bias_sb = bias_pool.tile([128, LK // 128, QC], DT,
                                         tag="bias", name=f"bias_{qc}_{h}")
                for g in range((nk + 3) // 4):
                    k0, k1 = g * 4, min(nk, g * 4 + 4)
                    dma = nc.gpsimd.dma_start(
                        out=bias_sb[:, k0:k1, :],
                        in_=biasT[h, k0 * 128:k1 * 128,
                                  qc * QC:(qc + 1) * QC].rearrange(
                            "(ko p) q -> p ko q", p=128))
                    if not_before is not None:
                        add_dep_helper(dma.ins, not_before,
                                       reason="keep bias off startup HBM")
                bias_tiles[(qc, h)] = bias_sb

            # persistent activations
            wo_sb = ppool.tile([128, NH, D], DT)      # [hd_in, h, d_out]
            qT_sb = ppool.tile([128, NH, LQ], DT)     # [dh, h, q]
            kT_sb = ppool.tile([128, LK], DT)         # [dh, k]
            v_sb = ppool.tile([128, LK // 128, DH], DT)   # [k_in, k_blk, dh]
            un_sb = ppool.tile([128, NH, LQ], DT)     # normalized U_T

            # ---------------- stage 1: projections ----------------
            with tc.tile_pool(name="w1", bufs=1) as w1pool, \
                 tc.tile_pool(name="slab", bufs=4) as slab_pool, \
                 tc.tile_pool(name="proj_ps", bufs=3, space="PSUM") as proj_ps, \
                 tc.tile_pool(name="vtr_ps", bufs=2, space="PSUM") as vtr_ps, \
                 nc.named_scope("proj"):
                # all weights ride the SP HWDGE ring (in need order) while
                # the SWDGE ring carries only hidden-state slabs and bias
                wk_sb = w1pool.tile([128, KO, DH], DT)
                nc.sync.dma_start(
                    out=wk_sb[:], in_=wk.rearrange("(ko p) d -> p ko d", p=128))
                wv_sb = w1pool.tile([128, KO, DH], DT)
                nc.sync.dma_start(
                    out=wv_sb[:], in_=wv.rearrange("(ko p) d -> p ko d", p=128))
                vT_sb = w1pool.tile([128, LK], DT)        # [dh, k]
                wq_sb = w1pool.tile([128, KO, NH * DH], DT)
                wq_dmas = []
                for g in range(4):
                    wq_dmas.append(nc.sync.dma_start(
                        out=wq_sb[:, g * 8:(g + 1) * 8, :],
                        in_=wq[g * 1024:(g + 1) * 1024, :].rearrange(
                            "(ko p) d -> p ko d", p=128)))
                # interleaved kv/q steps: spreads the SWDGE slab traffic
                # evenly so the ring never starves the PE
                early_markers = {}

                def kv_step(s):
                    kt_ps = proj_ps.tile([128, QC], FP32, tag="pps",
                                         name=f"ktps_{s}")
                    vt_ps = proj_ps.tile([128, QC], FP32, tag="pps2",
                                         name=f"vtps_{s}")
                    for quarter in range(4):
                        slab = slab_pool.tile([128, KO // 4, QC], DT,
                                              tag="slab", name=f"kvslab_{s}_{quarter}")
                        r0 = quarter * 1024
                        dma = nc.gpsimd.dma_start(
                            out=slab[:],
                            in_=hkvT[r0:r0 + 1024,
                                     s * QC:(s + 1) * QC].rearrange(
                                "(ko p) q -> p ko q", p=128))
                        if s == 0 and quarter == 2:
                            early_markers['kv0q2'] = dma.ins
                        for kl in range(KO // 4):
                            ko = quarter * (KO // 4) + kl
                            nc.tensor.matmul(kt_ps[:], lhsT=wk_sb[:, ko, :],
                                             rhs=slab[:, kl, :],
                                             start=(ko == 0), stop=(ko == KO - 1))
                        for kl in range(KO // 4):
                            ko = quarter * (KO // 4) + kl
                            nc.tensor.matmul(vt_ps[:], lhsT=wv_sb[:, ko, :],
                                             rhs=slab[:, kl, :],
                                             start=(ko == 0), stop=(ko == KO - 1))
                    nc.scalar.copy(out=kT_sb[:, s * QC:(s + 1) * QC], in_=kt_ps[:])
                    return nc.scalar.copy(out=vT_sb[:, s * QC:(s + 1) * QC],
                                          in_=vt_ps[:])

                def q_step(s):
                    q_pss = [proj_ps.tile([128, QC], FP32,
                                          tag=("pps" if h % 2 == 0 else "pps2"),
                                          name=f"qps_{s}_{h}")
                             for h in range(NH)]
                    for quarter in range(4):
                        slab = slab_pool.tile([128, KO // 4, QC], DT,
                                              tag="slab", name=f"qslab_{s}_{quarter}")
                        r0 = quarter * 1024
                        nc.gpsimd.dma_start(
                            out=slab[:],
                            in_=hqT[r0:r0 + 1024,
                                    s * QC:(s + 1) * QC].rearrange(
                                "(ko p) q -> p ko q", p=128))
                        for h in range(NH):
                            for kl in range(KO // 4):
                                ko = quarter * (KO // 4) + kl
                                nc.tensor.matmul(
                                    q_pss[h][:],
                                    lhsT=wq_sb[:, ko, h * DH:(h + 1) * DH],
                                    rhs=slab[:, kl, :],
                                    start=(ko == 0), stop=(ko == KO - 1))
                    for h in range(NH):
                        nc.scalar.copy(out=qT_sb[:, h, s * QC:(s + 1) * QC],
                                       in_=q_pss[h][:])

                def v_transpose():
                    make_identity(nc, ident_sb[:])
                    for blk in range(LK // 128):
                        tp = vtr_ps.tile([128, 128], DT, tag="vtr",
                                         name=f"vtr_{blk}")
                        nc.tensor.transpose(
                            tp[:], vT_sb[:, blk * 128:(blk + 1) * 128],
                            ident_sb[:])
                        nc.scalar.copy(out=v_sb[:, blk, :], in_=tp[:])

                kv_step(0)
                for dma in wq_dmas:
                    add_dep_helper(dma.ins, early_markers['kv0q2'],
                                   reason="keep wq off the startup HBM burst")
                q_step(0)
                kv_step(1)
                q_step(1)
                marker = kv_step(2).ins
                q_step(2)
                kv_step(3)
                v_transpose()
                for h in range(NH):
                    # wo/bias are not needed until the attention phase; the
                    # explicit dep keeps the scheduler from hoisting their
                    # transfers into the HBM-critical startup window
                    dma = nc.sync.dma_start(
                        out=wo_sb[:, h, :],
                        in_=wo[h * 128:(h + 1) * 128, :].rearrange(
                            "(hh p) d -> p hh d", p=128))
                    add_dep_helper(dma.ins, marker,
                                   reason="keep wo off startup HBM")
                for h in range(NH):
                    fetch_bias(0, h, not_before=marker)
                q_step(3)

            # ------- stage 2+3: attention interleaved with out-proj -------
            # unnormalized U_T and row-sums staged to SBUF by ACT so the
            # reciprocal/normalize never sits on the DVE critical path

            inv2 = float(DH ** -0.5)
            with tc.tile_pool(name="att_sb", bufs=6) as att_sb, \
                 tc.tile_pool(name="us_sb", bufs=8) as us_pool, \
                 tc.tile_pool(name="sc_ps", bufs=3, space="PSUM") as sc_pool, \
                 tc.tile_pool(name="acc_ps", bufs=2, space="PSUM") as acc_pool, \
                 tc.tile_pool(name="osb", bufs=2) as out_pool, \
                 tc.tile_pool(name="ops", bufs=2, space="PSUM") as out_ps, \
                 nc.named_scope("attn_wo"):

                def emit_attn_unit(qc, h, filler=None):
                    nk = nk_per_qc[qc]
                    if (qc, h) not in bias_tiles:
                        fetch_bias(qc, h)
                    bias_sb = bias_tiles.pop((qc, h))
                    u_ps = acc_pool.tile([128, QC], FP32, tag="ups",
                                         name=f"ups_{qc}_{h}")
                    s_ps = acc_pool.tile([128, QC], FP32, tag="sps", bufs=1,
                                         name=f"sps_{qc}_{h}")
                    e_tiles = {}

                    def off_of(kc):
                        # columns left of the first live one are fully
                        # masked for this key block — skip them
                        return off_table[qc][kc]

                    # software pipeline: score/bias/exp for kc emitted before
                    # the accumulation matmuls of kc-1
                    for kc in range(nk + 1):
                        if kc < nk:
                            off = off_of(kc)
                            sc_ps = sc_pool.tile([128, QC], FP32, tag="scps",
                                                 name=f"scps_{qc}_{h}_{kc}")
                            nc.tensor.matmul(
                                sc_ps[:, off:],
                                lhsT=kT_sb[:, kc * 128:(kc + 1) * 128],
                                rhs=qT_sb[:, h, qc * QC + off:(qc + 1) * QC],
                                start=True, stop=True)
                            t_sb = att_sb.tile([128, QC], FP32, tag="tsb",
                                               name=f"tsb_{qc}_{h}_{kc}")
                            # t = score * dh^-0.5 + bias
                            nc.vector.scalar_tensor_tensor(
                                out=t_sb[:, off:], in0=sc_ps[:, off:],
                                scalar=inv2,
                                in1=bias_sb[:, kc, off:],
                                op0=AluOpType.mult,
                                op1=AluOpType.add)
                            e_sb = att_sb.tile([128, QC], DT, tag="esb",
                                               name=f"esb_{qc}_{h}_{kc}")
                            nc.scalar.activation(
                                out=e_sb[:, off:], in_=t_sb[:, off:],
                                func=mybir.ActivationFunctionType.Exp)
                            e_tiles[kc] = e_sb
                        if kc == min(1, nk - 1) and filler is not None:
                            # PE chews the out-projection chunk here while
                            # DVE/ACT produce the first exp tiles — hides the
                            # score->exp latency at the unit boundary
                            filler()
                        if kc >= 1:
                            off = off_of(kc - 1)
                            e_prev = e_tiles.pop(kc - 1)
                            nc.tensor.matmul(u_ps[:, off:],
                                             lhsT=v_sb[:, kc - 1, :],
                                             rhs=e_prev[:, off:],
                                             start=(kc == 1), stop=(kc == nk),
                                             skip_group_check=True)
                            nc.tensor.matmul(s_ps[:, off:], lhsT=ones_sb[:],
                                             rhs=e_prev[:, off:],
                                             start=(kc == 1), stop=(kc == nk),
                                             skip_group_check=True)
                    nxt = (qc, h + 1) if h + 1 < NH else (qc + 1, 0)
                    if nxt[0] < n_s and nxt not in bias_tiles and nxt != (0, 0):
                        fetch_bias(*nxt)
                    u_sb = us_pool.tile([128, QC], DT, tag="usb",
                                        name=f"usb_{qc}_{h}")
                    s_sb = us_pool.tile([128, QC], DT, tag="ssb",
                                        name=f"ssb_{qc}_{h}")
                    nc.scalar.copy(out=u_sb[:], in_=u_ps[:])
                    nc.scalar.copy(out=s_sb[:], in_=s_ps[:])
                    us_tiles[(qc, h)] = (u_sb, s_sb)

                def emit_norm_piece(qc, h):
                    qsl = slice(qc * QC, (qc + 1) * QC)
                    u_sb, s_sb = us_tiles.pop((qc, h))
                    sinv_sb = att_sb.tile([128, QC], DT, tag="sinv",
                                          name=f"sinv_{qc}_{h}")
                    # 1/S as exp(-ln S) on ACT: Ln and Exp share the
                    # natural_log_exp table set (no table switch), and the
                    # DVE iterative reciprocal would stall the STT pipe
                    lns_sb = att_sb.tile([128, QC], FP32, tag="lns",
                                         name=f"lns_{qc}_{h}")
                    nc.scalar.activation(
                        out=lns_sb[:], in_=s_sb[:],
                        func=mybir.ActivationFunctionType.Ln)
                    nc.scalar.activation(
                        out=sinv_sb[:], in_=lns_sb[:],
                        func=mybir.ActivationFunctionType.Exp,
                        scale=-1.0)
                    nc.vector.tensor_mul(
                        un_sb[:, h, qsl], u_sb[:], sinv_sb[:])

                def emit_wo_unit(m):
                    o_sb = out_pool.tile([128, D], DT, tag="osb",
                                         name=f"osb_{m}")
                    for n in range(D // QC):
                        o_ps = out_ps.tile([128, QC], FP32, tag="ops",
                                           name=f"ops_{m}_{n}")
                        for h in range(NH):
                            nc.tensor.matmul(
                                o_ps[:],
                                lhsT=un_sb[:, h, m * 128:(m + 1) * 128],
                                rhs=wo_sb[:, h, n * QC:(n + 1) * QC],
                                start=(h == 0), stop=(h == NH - 1))
                        nc.scalar.copy(out=o_sb[:, n * QC:(n + 1) * QC],
                                       in_=o_ps[:])
                    nc.scalar.dma_start(out=out[m * 128:(m + 1) * 128, :],
                                        in_=o_sb[:])

                # interleave: attention units stream with 1-unit-delayed
                # norm pieces (keeps the reciprocal off the DVE critical
                # path); wo chunks for qc are woven in as soon as norm(qc)
                # completes, two per unit during the final qc to shrink the
                # tail
                us_tiles = {}
                wo_queue = []
                units = [(qc, h) for qc in range(n_s) for h in range(NH)]
                for i, (qc, h) in enumerate(units):
                    def filler():
                        if wo_queue:
                            emit_wo_unit(wo_queue.pop(0))
                    emit_attn_unit(qc, h, filler=filler if i >= 1 else None)
                    if i >= 1:
                        pqc, ph = units[i - 1]
                        emit_norm_piece(pqc, ph)
                        if ph == NH - 1:
                            wo_queue.extend(range(4 * pqc, 4 * pqc + 4))
                    if qc == n_s - 1 and wo_queue:
                        emit_wo_unit(wo_queue.pop(0))
                emit_norm_piece(*units[-1])
                wo_queue.extend(range(4 * (n_s - 1), 4 * n_s))
                for m in wo_queue:
                    emit_wo_unit(m)
    _split_waits(nc)
    return nc


def _split_waits(nc):
    """Walrus in this container accepts at most one sync wait per
    instruction: hoist extra waits onto same-engine nops placed directly
    before the instruction (identical semantics — the engine stream
    blocks on each in order)."""
    import bass_rust
    ctr = 0
    for f in nc.m.functions:
        for bb in f.blocks:
            new = []
            for inst in bb.instructions:
                si = inst.sync_info
                if si is not None and si.on_wait and len(si.on_wait) > 1:
                    waits = list(si.on_wait)
                    for w in waits[:-1]:
                        nop = bass_rust.InstNoOp(name=f"waitnop-{ctr}",
                                                 engine=inst.engine)
                        ctr += 1
                        nop.sync_info = bass_rust.SyncInfo(on_wait=[w],
                                                           on_update=[])
                        new.append(nop)
                    si.on_wait = waits[-1:]
                    inst.sync_info = si
                new.append(inst)
            bb.instructions = new


_CACHE = {}


def kernel(hidden_q, hidden_kv, attention_mask, position_bias, Wq, Wk, Wv, Wo):
    hq = np.asarray(hidden_q, dtype=np.float32)[0]      # [2048, 4096]
    hkv = np.asarray(hidden_kv, dtype=np.float32)[0]
    mask = np.asarray(attention_mask)[0]                # [2048, 2048] bool
    pb = np.asarray(position_bias, dtype=np.float32)    # [32, 2048, 2048]
    Wq = np.asarray(Wq, dtype=np.float32)
    Wk = np.asarray(Wk, dtype=np.float32)
    Wv = np.asarray(Wv, dtype=np.float32)
    Wo = np.asarray(Wo, dtype=np.float32)

    # additive mask, transposed to [k, q]
    negT = np.where(mask, np.float32(0.0), np.float32(NEG)).T

    # which 128-key chunks are live for each 512-query chunk
    n_s = LQ // QC
    nk_per_qc = []
    off_table = []
    for qc in range(n_s):
        cols = negT[:, qc * QC:(qc + 1) * QC]            # [2048k, 512q]
        live = 0
        offs = []
        for kc in range(LK // 128):
            blk_live = np.any(cols[kc * 128:(kc + 1) * 128] != np.float32(NEG),
                              axis=0)                    # per q column
            if blk_live.any() and kc > 0:
                live = kc + 1
                offs.append(int(np.argmax(blk_live)) // 128 * 128)
            else:
                # kc == 0 must write the full psum width (accumulator init)
                live = max(live, kc + 1) if blk_live.any() else live
                offs.append(0)
        nk_per_qc.append(live)
        off_table.append(offs)
    key = (tuple(nk_per_qc), tuple(tuple(o) for o in off_table))

    if key not in _CACHE:
        _CACHE[key] = build_graph(nk_per_qc, off_table)
    nc = _CACHE[key]

    hqT = np.ascontiguousarray(hq.T).astype(BF16)        # [4096, 2048]
    hkvT = np.ascontiguousarray(hkv.T).astype(BF16)

    in_maps = []
    for i in range(N_CORES):
        bT = np.transpose(pb[NH * i:NH * (i + 1)], (0, 2, 1))  # [4, k, q]
        biasT = (bT + negT[None]).astype(BF16)
        in_maps.append({
            "hqT": hqT,
            "hkvT": hkvT,
            "wq": np.ascontiguousarray(Wq[:, i * NH * DH:(i + 1) * NH * DH]).astype(BF16),
            "wk": np.ascontiguousarray(Wk[:, i * DH:(i + 1) * DH]).astype(BF16),
            "wv": np.ascontiguousarray(Wv[:, i * DH:(i + 1) * DH]).astype(BF16),
            "wo": np.ascontiguousarray(Wo[i * NH * DH:(i + 1) * NH * DH, :]).astype(BF16),
            "biasT": biasT,
        })

    res = run_bass_kernel_spmd(nc, in_maps, list(range(N_CORES)))
    kernel.last_results = res

    acc = np.zeros((LQ, D), dtype=np.float32)
    for i in range(N_CORES):
        acc += res.results[i]["out"].astype(np.float32)
    return acc[None]
